# revision 17
# baseline (speedup 1.0000x reference)
"""Trainium2 Bass kernel for nn_DualOutputMoE.

Math: the reference collapses the whole MoE into a single [1,1,H] vector:
    acc = sum_e combine[:,e] @ (gelu(x @ W1[e] + b1[e]) @ W2[e] + b2[e])
    out = acc / total_weight
Since combine is applied *linearly* after the gelu, we contract it with the
gelu activations first:
    u_e  = combine[:,e] @ gelu(x @ W1[e] + b1[e])          # [F]
    acc  = sum_e (u_e @ W2[e] + combine[:,e].sum() * b2[e])
which turns the second [T,F]@[F,H] matmul into an [F]@[F,H] matvec — that
matvec (0.1% of the FLOPs) plus the b2 term runs on the host in the
finalize step, so the device only computes mm1 + gelu + combine.

Routing/top-k/gather is host-side (as in the baseline).  Tokens routed to
expert e are gathered into per-expert tiles of 128; each core gets 2 expert
slots with static capacities (nt0, nt1) tiles.  Experts are assigned to
slots largest-first so capacities stay tight; the ragged tail beyond a
slot's capacity (< 1 tile per expert, ~2-3% of tokens) is evaluated
exactly on the host and added into u during finalize.

Device kernel (per core, SPMD over 8 cores): mm1 runs in fp8(e4m3) with
DoubleRow (measured 2x over bf16); W1 is pre-scaled by 32 into fp8's
normal range and the gelu activation un-scales.  PSUM accumulation is fp32;
gelu output / combine weights are bf16.

Per f-chunk-PAIR (1024 wide, 2 PSUM banks — halves the count of ACT gelu
ops, which run ~1.1us each on [128,1024]):
  mm1:  psA[tok128, 0:512 | 512:1024] += xgDR[h].T @ W1DR[h, half]
  gelu: G[128,1024] = gelu(psA / 32)           (ACT, psum->sbuf bf16)
  cmm:  psB[1, half] += c[tok128, 1].T @ G[:, half]  (per token-tile, lag-1)
  u[fp*1024:...] = psB                          (DVE copy; DMA out at end)
"""

import sys
import math

if "/opt/trn_rl_repo" not in sys.path:
    sys.path.insert(0, "/opt/trn_rl_repo")

import numpy as np
import ml_dtypes
from scipy.special import erf

import concourse.bass as bass
import concourse.tile as tile
from concourse import bacc, mybir
from concourse.bass_utils import run_bass_kernel_spmd

BF16 = ml_dtypes.bfloat16
FP8 = ml_dtypes.float8_e4m3
N_CORES = 8
E = 16
EPC = E // N_CORES  # expert slots per core
H = 1024
F = 4096
TOP_K = 2
KH2 = H // 256  # 4 fp8-DoubleRow k-tiles (256 contraction per tile)
FCP = 4  # f-chunk PAIRS of 1024

W1_SCALE = 32.0  # pre-scale W1 into fp8's normal range; gelu un-scales
OVERFLOW_CAP = 160  # max ragged-tail tokens per expert evaluated on host

_compiled_cache = {}


def _build(nts: tuple, has_b1: bool = False, reps: int = 1):
    """Build + compile the SPMD device program; slot s holds nts[s] token
    tiles.  reps > 1 wraps the body in a hardware For_i loop (timing)."""
    key = (nts, has_b1, reps)
    if key in _compiled_cache:
        return _compiled_cache[key]

    nc = bacc.Bacc("TRN2", target_bir_lowering=False, debug=False)
    f32 = mybir.dt.float32
    bf16 = mybir.dt.bfloat16
    fp8 = mybir.dt.float8e4

    Cs = [nt * 128 for nt in nts]

    xg_d = [
        nc.dram_tensor(f"xg{s}", [KH2, 128, 2, Cs[s]], fp8, kind="ExternalInput").ap()
        for s in range(EPC)
    ]
    w1_d = nc.dram_tensor(
        "w1", [EPC, FCP, KH2, 128, 2048], fp8, kind="ExternalInput"
    ).ap()
    cw_d = [
        nc.dram_tensor(f"cw{s}", [1, Cs[s]], bf16, kind="ExternalInput").ap()
        for s in range(EPC)
    ]
    u_d = nc.dram_tensor("u", [EPC, F], f32, kind="ExternalOutput").ap()
    if has_b1:
        b1_d = nc.dram_tensor("b1", [EPC, F], bf16, kind="ExternalInput").ap()

    with tile.TileContext(nc) as tc:
        with (
            tc.tile_pool(name="xg", bufs=1) as xg_pool,
            tc.tile_pool(name="cw", bufs=1) as cw_pool,
            tc.tile_pool(name="w1", bufs=3) as w1_pool,
            tc.tile_pool(name="g", bufs=8) as g_pool,
            tc.tile_pool(name="u", bufs=1) as u_pool,
            tc.tile_pool(name="psA", bufs=2, space="PSUM") as psA_pool,
            tc.tile_pool(name="psB", bufs=2, space="PSUM") as psB_pool,
        ):
            xg_sb, cw_sb, u8, b1_sb = [], [], [], []
            ones_sb = None
            if has_b1:
                ones_sb = cw_pool.tile([1, 128], bf16, tag="ones", name="ones")
                nc.vector.memset(ones_sb[:], 1.0)
                for s in range(EPC):
                    b1_t = cw_pool.tile([1, F], bf16, tag=f"b1{s}", name=f"b1{s}")
                    nc.sync.dma_start(b1_t[:], b1_d[s : s + 1, :])
                    b1_sb.append(b1_t)
            for s in range(EPC):
                xg_sb.append(
                    xg_pool.tile([128, KH2, 2, Cs[s]], fp8, tag=f"xg{s}", name=f"xg{s}")
                )
                cw_sb.append(
                    cw_pool.tile([128, nts[s]], bf16, tag=f"cw{s}", name=f"cw{s}")
                )
                u8.append(u_pool.tile([1, F], f32, tag=f"u{s}", name=f"u{s}"))

            def load_xg(s, kt):
                nc.sync.dma_start(xg_sb[s][:, kt, :, :], xg_d[s][kt])

            def load_cw(s):
                nc.sync.dma_start(
                    cw_sb[s][:], cw_d[s][0].rearrange("(t p) -> p t", p=128)
                )

            def load_w1(s, fp, split=False):
                w1_t = w1_pool.tile([128, KH2, 2048], fp8, name="w1t")
                if split:  # per-k-tile DMAs: first mm waits on one tile only
                    for kt in range(KH2):
                        nc.sync.dma_start(w1_t[:, kt, :], w1_d[s, fp, kt])
                else:
                    nc.sync.dma_start(w1_t[:], w1_d[s, fp].rearrange("k p n -> p k n"))
                return w1_t

            cmm_q = []  # (s, fp, tt, g_tile) pending combine matmuls
            psB_cur = {}

            def pop_cmm():
                if not cmm_q:
                    return
                s, fp, tt, g_t = cmm_q.pop(0)
                for half in range(2):
                    nc.tensor.matmul(
                        psB_cur[(s, fp)][:, half * 512 : (half + 1) * 512],
                        lhsT=cw_sb[s][:, tt : tt + 1],
                        rhs=g_t[:, half * 512 : (half + 1) * 512],
                        start=(tt == 0),
                        stop=(tt == nts[s] - 1),
                    )
                if tt == nts[s] - 1:
                    finish_pair(s, fp)

            def finish_pair(s, fp):
                psB = psB_cur.pop((s, fp))
                nc.vector.tensor_copy(u8[s][:, fp * 1024 : (fp + 1) * 1024], psB[:])

            def mm1_pair_block(s, fp, w1_t):
                """mm1s for one f-chunk pair; one [128,1024] gelu per tt;
                cmms pop with lag-1 so PE never waits on ACT."""
                psB_cur[(s, fp)] = psB_pool.tile([1, 1024], f32, name="psB")
                for tt in range(nts[s]):
                    psA = psA_pool.tile([128, 1024], f32, name="psA")
                    tsl = slice(tt * 128, (tt + 1) * 128)
                    for kt in range(KH2):
                        for half in range(2):
                            hsl = slice(half * 512, (half + 1) * 512)
                            rhs = w1_t[:, kt, :].rearrange("p (j n) -> p j n", j=2)[
                                :, :, hsl
                            ]
                            nc.tensor.matmul(
                                psA[:, hsl],
                                lhsT=xg_sb[s][:, kt, :, tsl],
                                rhs=rhs,
                                start=(kt == 0),
                                stop=(kt == KH2 - 1) and not has_b1,
                                perf_mode=mybir.MatmulPerfMode.DoubleRow,
                            )
                    if has_b1:
                        for half in range(2):
                            hsl = slice(half * 512, (half + 1) * 512)
                            nc.tensor.matmul(
                                psA[:, hsl],
                                lhsT=ones_sb[:],
                                rhs=b1_sb[s][
                                    :,
                                    fp * 1024 + half * 512 : fp * 1024
                                    + (half + 1) * 512,
                                ],
                                start=False,
                                stop=True,
                            )
                    pop_cmm()  # previous tt's cmms (their gelu is long done)
                    g_t = g_pool.tile([128, 1024], bf16, name="gt")
                    nc.scalar.activation(
                        g_t[:],
                        psA[:],
                        mybir.ActivationFunctionType.Gelu,
                        scale=1.0 / W1_SCALE,
                    )
                    cmm_q.append((s, fp, tt, g_t))

            def emit_body():
                # startup: only slot0's k0 slice + w1 pair0 k0 gate the 1st mm
                load_xg(0, 0)
                load_cw(0)
                w1_next = load_w1(0, 0, split=True)
                for kt in range(1, KH2):
                    load_xg(0, kt)
                for s in range(EPC):
                    for fp in range(FCP):
                        if s == 0 and fp < KH2:  # spread slot1 input loads out
                            load_xg(1, fp)
                            if fp == 0:
                                load_cw(1)
                        w1_cur = w1_next
                        if (s, fp) != (EPC - 1, FCP - 1):
                            ns, nf = (s, fp + 1) if fp + 1 < FCP else (s + 1, 0)
                            w1_next = load_w1(ns, nf)
                        mm1_pair_block(s, fp, w1_cur)
                while cmm_q:
                    pop_cmm()
                for s in range(EPC):
                    nc.sync.dma_start(u_d[s : s + 1, :], u8[s][:])

            if reps > 1:
                # body >256 insts/engine: back-edge branch misses IRAM
                # (~3-4us/iter) without prefetch hints on the big engines
                with tc.For_i(
                    0,
                    reps,
                    1,
                    hint_engines=(mybir.EngineType.PE, mybir.EngineType.SP),
                ):
                    emit_body()
            else:
                emit_body()

    nc.compile()
    _compiled_cache[key] = nc
    return nc


def _gelu_exact(y):
    return 0.5 * y * (1.0 + erf(y / np.sqrt(2.0)))


def _prep_inputs(input_tensor, Wg, bg, W1, b1, W2, b2):
    """Host-side gating, top-k, expert->slot assignment, gather, fp8
    conversion.  Returns (in_maps, meta)."""
    B, S, _ = input_tensor.shape
    T = B * S
    x = np.ascontiguousarray(input_tensor.reshape(T, H)).astype(np.float32)

    scores = x @ Wg.astype(np.float32) + bg.astype(np.float32)
    order = np.argsort(-scores, axis=1, kind="stable")
    top_i = order[:, :TOP_K]
    top_v = np.take_along_axis(scores, top_i, axis=1).astype(np.float64)
    ex = np.exp(top_v - top_v.max(axis=1, keepdims=True))
    top_w = ex / ex.sum(axis=1, keepdims=True)
    total_weight = float(top_w.sum())

    flat_e = top_i.ravel()
    flat_t = np.repeat(np.arange(T), TOP_K)
    flat_w = top_w.ravel()
    srt = np.argsort(flat_e, kind="stable")
    flat_e, flat_t, flat_w = flat_e[srt], flat_t[srt], flat_w[srt]
    counts = np.bincount(flat_e, minlength=E)
    starts = np.concatenate([[0], np.cumsum(counts)])
    csum = np.zeros(E, dtype=np.float64)
    for e in range(E):
        csum[e] = flat_w[starts[e] : starts[e + 1]].sum()

    # Slot assignment: sort experts by count desc; slot0 = 8 largest (one
    # per core), slot1 = 8 smallest.  Slot capacity = max floor-tiles over
    # its experts, bumped up while any expert's ragged tail (host-evaluated)
    # would exceed OVERFLOW_CAP tokens.
    o = np.argsort(-counts, kind="stable")
    slot_exp = [
        [int(o[i]) for i in range(N_CORES)],
        [int(o[E - 1 - i]) for i in range(N_CORES)],
    ]
    nts = []
    for s in range(EPC):
        cmax = max(int(counts[e]) for e in slot_exp[s])
        nt = max(1, cmax // 128)
        while cmax - nt * 128 > OVERFLOW_CAP:
            nt += 1
        nts.append(nt)
    nts = tuple(nts)

    # Gather tokens per (core, slot); overflow tokens -> host list.
    xg = [np.zeros((N_CORES, KH2, 128, 2, nts[s] * 128), dtype=FP8) for s in range(EPC)]
    cw = [np.zeros((N_CORES, 1, nts[s] * 128), dtype=BF16) for s in range(EPC)]
    overflow = []  # (expert, token_idx_array, weight_array)
    for s in range(EPC):
        cap = nts[s] * 128
        for i in range(N_CORES):
            e = slot_exp[s][i]
            lo, hi = starts[e], starts[e + 1]
            n = min(hi - lo, cap)
            toks = flat_t[lo : lo + n]
            # DoubleRow interleave: h = kt*256 + p*2 + j
            xt = x[toks].T.astype(FP8).reshape(KH2, 128, 2, n)
            xg[s][i, :, :, :, :n] = xt
            cw[s][i, 0, :n] = flat_w[lo : lo + n].astype(BF16)
            if hi - lo > cap:
                overflow.append(
                    (e, flat_t[lo + cap : hi], flat_w[lo + cap : hi].copy())
                )

    # W1 per (core, slot): fp8 DoubleRow fc-pair-major layout
    w1c = np.zeros((N_CORES, EPC, FCP, KH2, 128, 2048), dtype=FP8)
    for s in range(EPC):
        for i in range(N_CORES):
            e = slot_exp[s][i]
            t = (W1[e].reshape(KH2, 128, 2, FCP, 1024) * W1_SCALE).astype(FP8)
            w1c[i, s] = t.transpose(3, 0, 1, 2, 4).reshape(FCP, KH2, 128, 2048)

    has_b1 = bool(np.any(b1))
    in_maps = []
    for i in range(N_CORES):
        m = {"w1": np.ascontiguousarray(w1c[i])}
        for s in range(EPC):
            m[f"xg{s}"] = np.ascontiguousarray(xg[s][i])
            m[f"cw{s}"] = np.ascontiguousarray(cw[s][i])
        if has_b1:
            m["b1"] = np.ascontiguousarray(
                np.stack(
                    [b1[slot_exp[s][i]] * W1_SCALE for s in range(EPC)]
                ).astype(BF16)
            )
        in_maps.append(m)

    meta = {
        "nts": nts,
        "slot_exp": slot_exp,
        "csum": csum,
        "total_weight": total_weight,
        "overflow": overflow,
        "x": x,
        "b1": np.asarray(b1, dtype=np.float64),
        "has_b1": has_b1,
    }
    return in_maps, meta


def _finalize(results, meta, W1, W2, b2):
    """acc = sum_e u_e @ W2[e] (+ host ragged tail) + csum @ b2; /tw."""
    slot_exp = meta["slot_exp"]
    u = np.zeros((E, F), dtype=np.float64)
    for i in range(N_CORES):
        for s in range(EPC):
            u[slot_exp[s][i]] += results[i]["u"][s].astype(np.float64)
    for e, toks, w in meta["overflow"]:
        y = meta["x"][toks].astype(np.float64) @ W1[e].astype(np.float64)
        if meta["has_b1"]:
            y += meta["b1"][e]
        u[e] += w @ _gelu_exact(y)
    acc = np.einsum("ef,efh->h", u, W2.astype(np.float64))
    acc += meta["csum"] @ b2.astype(np.float64)
    return (acc / meta["total_weight"]).reshape(1, 1, H).astype(np.float32)


def kernel(input_tensor, Wg, bg, W1, b1, W2, b2):
    in_maps, meta = _prep_inputs(input_tensor, Wg, bg, W1, b1, W2, b2)
    nc = _build(meta["nts"], meta["has_b1"])
    res = run_bass_kernel_spmd(nc, in_maps, core_ids=list(range(N_CORES)))
    return _finalize(res.results, meta, W1, W2, b2)


# revision 18
# speedup vs baseline: 1.9162x; 1.9162x over previous
"""Trainium2 Bass kernel for nn_DualOutputMoE.

Math: the reference collapses the whole MoE into a single [1,1,H] vector:
    acc = sum_e combine[:,e] @ (gelu(x @ W1[e] + b1[e]) @ W2[e] + b2[e])
    out = acc / total_weight
Since combine is applied *linearly* after the gelu, we contract it with the
gelu activations first:
    u_e  = combine[:,e] @ gelu(x @ W1[e] + b1[e])          # [F]
    acc  = sum_e (u_e @ W2[e] + combine[:,e].sum() * b2[e])
which turns the second [T,F]@[F,H] matmul into an [F]@[F,H] matvec — that
matvec (0.1% of the FLOPs) plus the b2 term runs on the host in the
finalize step, so the device only computes mm1 + gelu + combine.

Routing/top-k/gather is host-side (as in the baseline).  Tokens routed to
expert e are gathered into per-expert tiles of 128; each core gets 2 expert
slots with static capacities (nt0, nt1) tiles.  Experts are assigned to
slots largest-first so capacities stay tight; the ragged tail beyond a
slot's capacity (< 1 tile per expert, ~2-3% of tokens) is evaluated
exactly on the host and added into u during finalize.

Device kernel (per core, SPMD over 8 cores): mm1 runs in fp8(e4m3) with
DoubleRow (measured 2x over bf16); W1 is pre-scaled by 32 into fp8's
normal range and the gelu activation un-scales.  PSUM accumulation is fp32;
gelu output / combine weights are bf16.

Per f-chunk-PAIR (1024 wide, 2 PSUM banks — halves the count of ACT gelu
ops, which run ~1.1us each on [128,1024]):
  mm1:  psA[tok128, 0:512 | 512:1024] += xgDR[h].T @ W1DR[h, half]
  gelu: G[128,1024] = gelu(psA / 32)           (ACT, psum->sbuf bf16)
  cmm:  psB[1, half] += c[tok128, 1].T @ G[:, half]  (per token-tile, lag-1)
  u[fp*1024:...] = psB                          (DVE copy; DMA out at end)
"""

import sys
import math

if "/opt/trn_rl_repo" not in sys.path:
    sys.path.insert(0, "/opt/trn_rl_repo")

import numpy as np
import ml_dtypes
from scipy.special import erf

import concourse.bass as bass
import concourse.tile as tile
from concourse import bacc, mybir
from concourse.bass_utils import run_bass_kernel_spmd

BF16 = ml_dtypes.bfloat16
FP8 = ml_dtypes.float8_e4m3
N_CORES = 8
E = 16
EPC = E // N_CORES  # expert slots per core
H = 1024
F = 4096
TOP_K = 2
KH2 = H // 256  # 4 fp8-DoubleRow k-tiles (256 contraction per tile)
FCP = 4  # f-chunk PAIRS of 1024

W1_SCALE = 32.0  # pre-scale W1 into fp8's normal range; gelu un-scales
OVERFLOW_CAP = 160  # max ragged-tail tokens per expert evaluated on host

_compiled_cache = {}


def _build(nts: tuple, has_b1: bool = False, reps: int = 1):
    """Build + compile the SPMD device program; slot s holds nts[s] token
    tiles.  reps > 1 wraps the body in a hardware For_i loop (timing)."""
    key = (nts, has_b1, reps)
    if key in _compiled_cache:
        return _compiled_cache[key]

    nc = bacc.Bacc("TRN2", target_bir_lowering=False, debug=False)
    f32 = mybir.dt.float32
    bf16 = mybir.dt.bfloat16
    fp8 = mybir.dt.float8e4

    Cs = [nt * 128 for nt in nts]

    xg_d = [
        nc.dram_tensor(f"xg{s}", [KH2, 128, 2, Cs[s]], fp8, kind="ExternalInput").ap()
        for s in range(EPC)
    ]
    w1_d = nc.dram_tensor(
        "w1", [EPC, FCP, KH2, 128, 2048], fp8, kind="ExternalInput"
    ).ap()
    cw_d = [
        nc.dram_tensor(f"cw{s}", [1, Cs[s]], bf16, kind="ExternalInput").ap()
        for s in range(EPC)
    ]
    u_d = nc.dram_tensor("u", [EPC, F], f32, kind="ExternalOutput").ap()
    if has_b1:
        b1_d = nc.dram_tensor("b1", [EPC, F], bf16, kind="ExternalInput").ap()

    with tile.TileContext(nc) as tc:
        with (
            tc.tile_pool(name="xg", bufs=1) as xg_pool,
            tc.tile_pool(name="cw", bufs=1) as cw_pool,
            tc.tile_pool(name="w1", bufs=3) as w1_pool,
            tc.tile_pool(name="g", bufs=8) as g_pool,
            tc.tile_pool(name="u", bufs=1) as u_pool,
            tc.tile_pool(name="psA", bufs=2, space="PSUM") as psA_pool,
            tc.tile_pool(name="psB", bufs=2, space="PSUM") as psB_pool,
        ):
            xg_sb, cw_sb, u8, b1_sb = [], [], [], []
            ones_sb = None
            if has_b1:
                ones_sb = cw_pool.tile([1, 128], bf16, tag="ones", name="ones")
                nc.vector.memset(ones_sb[:], 1.0)
                for s in range(EPC):
                    b1_t = cw_pool.tile([1, F], bf16, tag=f"b1{s}", name=f"b1{s}")
                    nc.sync.dma_start(b1_t[:], b1_d[s : s + 1, :])
                    b1_sb.append(b1_t)
            for s in range(EPC):
                xg_sb.append(
                    xg_pool.tile([128, KH2, 2, Cs[s]], fp8, tag=f"xg{s}", name=f"xg{s}")
                )
                cw_sb.append(
                    cw_pool.tile([128, nts[s]], bf16, tag=f"cw{s}", name=f"cw{s}")
                )
                u8.append(u_pool.tile([1, F], f32, tag=f"u{s}", name=f"u{s}"))

            def load_xg(s, kt):
                nc.sync.dma_start(xg_sb[s][:, kt, :, :], xg_d[s][kt])

            def load_cw(s):
                nc.sync.dma_start(
                    cw_sb[s][:], cw_d[s][0].rearrange("(t p) -> p t", p=128)
                )

            def load_w1(s, fp, split=False):
                w1_t = w1_pool.tile([128, KH2, 2048], fp8, name="w1t")
                if split:  # per-k-tile DMAs: first mm waits on one tile only
                    for kt in range(KH2):
                        nc.sync.dma_start(w1_t[:, kt, :], w1_d[s, fp, kt])
                else:
                    nc.sync.dma_start(w1_t[:], w1_d[s, fp].rearrange("k p n -> p k n"))
                return w1_t

            cmm_q = []  # (s, fp, tt, g_tile) pending combine matmuls
            psB_cur = {}

            def pop_cmm():
                if not cmm_q:
                    return
                s, fp, tt, g_t = cmm_q.pop(0)
                for half in range(2):
                    nc.tensor.matmul(
                        psB_cur[(s, fp)][:, half * 512 : (half + 1) * 512],
                        lhsT=cw_sb[s][:, tt : tt + 1],
                        rhs=g_t[:, half * 512 : (half + 1) * 512],
                        start=(tt == 0),
                        stop=(tt == nts[s] - 1),
                    )
                if tt == nts[s] - 1:
                    finish_pair(s, fp)

            def finish_pair(s, fp):
                psB = psB_cur.pop((s, fp))
                nc.vector.tensor_copy(u8[s][:, fp * 1024 : (fp + 1) * 1024], psB[:])

            def mm1_pair_block(s, fp, w1_t):
                """mm1s for one f-chunk pair; one [128,1024] gelu per tt;
                cmms pop with lag-1 so PE never waits on ACT."""
                psB_cur[(s, fp)] = psB_pool.tile([1, 1024], f32, name="psB")
                for tt in range(nts[s]):
                    psA = psA_pool.tile([128, 1024], f32, name="psA")
                    tsl = slice(tt * 128, (tt + 1) * 128)
                    for kt in range(KH2):
                        for half in range(2):
                            hsl = slice(half * 512, (half + 1) * 512)
                            rhs = w1_t[:, kt, :].rearrange("p (j n) -> p j n", j=2)[
                                :, :, hsl
                            ]
                            nc.tensor.matmul(
                                psA[:, hsl],
                                lhsT=xg_sb[s][:, kt, :, tsl],
                                rhs=rhs,
                                start=(kt == 0),
                                stop=(kt == KH2 - 1) and not has_b1,
                                perf_mode=mybir.MatmulPerfMode.DoubleRow,
                            )
                    if has_b1:
                        for half in range(2):
                            hsl = slice(half * 512, (half + 1) * 512)
                            nc.tensor.matmul(
                                psA[:, hsl],
                                lhsT=ones_sb[:],
                                rhs=b1_sb[s][
                                    :,
                                    fp * 1024 + half * 512 : fp * 1024
                                    + (half + 1) * 512,
                                ],
                                start=False,
                                stop=True,
                            )
                    # lag-2 pop: give gelu(tt) two mm1 windows (2x872ns >
                    # 1147ns ACT) before its cmm, so PE never waits on ACT
                    if len(cmm_q) >= 2:
                        pop_cmm()
                    g_t = g_pool.tile([128, 1024], bf16, name="gt")
                    nc.scalar.activation(
                        g_t[:],
                        psA[:],
                        mybir.ActivationFunctionType.Gelu,
                        scale=1.0 / W1_SCALE,
                    )
                    cmm_q.append((s, fp, tt, g_t))

            def emit_body():
                # startup: only slot0's k0 slice + w1 pair0 k0 gate the 1st mm
                load_xg(0, 0)
                load_cw(0)
                w1_next = load_w1(0, 0, split=True)
                for kt in range(1, KH2):
                    load_xg(0, kt)
                for s in range(EPC):
                    for fp in range(FCP):
                        if s == 0 and fp < KH2:  # spread slot1 input loads out
                            load_xg(1, fp)
                            if fp == 0:
                                load_cw(1)
                        w1_cur = w1_next
                        if (s, fp) != (EPC - 1, FCP - 1):
                            ns, nf = (s, fp + 1) if fp + 1 < FCP else (s + 1, 0)
                            w1_next = load_w1(ns, nf)
                        mm1_pair_block(s, fp, w1_cur)
                while cmm_q:
                    pop_cmm()
                for s in range(EPC):
                    nc.sync.dma_start(u_d[s : s + 1, :], u8[s][:])

            if reps > 1:
                # body >256 insts/engine: back-edge branch misses IRAM
                # (~3-4us/iter) without prefetch hints on the big engines
                with tc.For_i(
                    0,
                    reps,
                    1,
                    hint_engines=(mybir.EngineType.PE, mybir.EngineType.SP),
                ):
                    emit_body()
            else:
                emit_body()

    nc.compile()
    _compiled_cache[key] = nc
    return nc


def _gelu_exact(y):
    return 0.5 * y * (1.0 + erf(y / np.sqrt(2.0)))


def _prep_inputs(input_tensor, Wg, bg, W1, b1, W2, b2):
    """Host-side gating, top-k, expert->slot assignment, gather, fp8
    conversion.  Returns (in_maps, meta)."""
    B, S, _ = input_tensor.shape
    T = B * S
    x = np.ascontiguousarray(input_tensor.reshape(T, H)).astype(np.float32)

    scores = x @ Wg.astype(np.float32) + bg.astype(np.float32)
    order = np.argsort(-scores, axis=1, kind="stable")
    top_i = order[:, :TOP_K]
    top_v = np.take_along_axis(scores, top_i, axis=1).astype(np.float64)
    ex = np.exp(top_v - top_v.max(axis=1, keepdims=True))
    top_w = ex / ex.sum(axis=1, keepdims=True)
    total_weight = float(top_w.sum())

    flat_e = top_i.ravel()
    flat_t = np.repeat(np.arange(T), TOP_K)
    flat_w = top_w.ravel()
    srt = np.argsort(flat_e, kind="stable")
    flat_e, flat_t, flat_w = flat_e[srt], flat_t[srt], flat_w[srt]
    counts = np.bincount(flat_e, minlength=E)
    starts = np.concatenate([[0], np.cumsum(counts)])
    csum = np.zeros(E, dtype=np.float64)
    for e in range(E):
        csum[e] = flat_w[starts[e] : starts[e + 1]].sum()

    # Slot assignment: sort experts by count desc; slot0 = 8 largest (one
    # per core), slot1 = 8 smallest.  Slot capacity = max floor-tiles over
    # its experts, bumped up while any expert's ragged tail (host-evaluated)
    # would exceed OVERFLOW_CAP tokens.
    o = np.argsort(-counts, kind="stable")
    slot_exp = [
        [int(o[i]) for i in range(N_CORES)],
        [int(o[E - 1 - i]) for i in range(N_CORES)],
    ]
    nts = []
    for s in range(EPC):
        cmax = max(int(counts[e]) for e in slot_exp[s])
        nt = max(1, cmax // 128)
        while cmax - nt * 128 > OVERFLOW_CAP:
            nt += 1
        nts.append(nt)
    nts = tuple(nts)

    # Gather tokens per (core, slot); overflow tokens -> host list.
    xg = [np.zeros((N_CORES, KH2, 128, 2, nts[s] * 128), dtype=FP8) for s in range(EPC)]
    cw = [np.zeros((N_CORES, 1, nts[s] * 128), dtype=BF16) for s in range(EPC)]
    overflow = []  # (expert, token_idx_array, weight_array)
    for s in range(EPC):
        cap = nts[s] * 128
        for i in range(N_CORES):
            e = slot_exp[s][i]
            lo, hi = starts[e], starts[e + 1]
            n = min(hi - lo, cap)
            toks = flat_t[lo : lo + n]
            # DoubleRow interleave: h = kt*256 + p*2 + j
            xt = x[toks].T.astype(FP8).reshape(KH2, 128, 2, n)
            xg[s][i, :, :, :, :n] = xt
            cw[s][i, 0, :n] = flat_w[lo : lo + n].astype(BF16)
            if hi - lo > cap:
                overflow.append(
                    (e, flat_t[lo + cap : hi], flat_w[lo + cap : hi].copy())
                )

    # W1 per (core, slot): fp8 DoubleRow fc-pair-major layout
    w1c = np.zeros((N_CORES, EPC, FCP, KH2, 128, 2048), dtype=FP8)
    for s in range(EPC):
        for i in range(N_CORES):
            e = slot_exp[s][i]
            t = (W1[e].reshape(KH2, 128, 2, FCP, 1024) * W1_SCALE).astype(FP8)
            w1c[i, s] = t.transpose(3, 0, 1, 2, 4).reshape(FCP, KH2, 128, 2048)

    has_b1 = bool(np.any(b1))
    in_maps = []
    for i in range(N_CORES):
        m = {"w1": np.ascontiguousarray(w1c[i])}
        for s in range(EPC):
            m[f"xg{s}"] = np.ascontiguousarray(xg[s][i])
            m[f"cw{s}"] = np.ascontiguousarray(cw[s][i])
        if has_b1:
            m["b1"] = np.ascontiguousarray(
                np.stack(
                    [b1[slot_exp[s][i]] * W1_SCALE for s in range(EPC)]
                ).astype(BF16)
            )
        in_maps.append(m)

    meta = {
        "nts": nts,
        "slot_exp": slot_exp,
        "csum": csum,
        "total_weight": total_weight,
        "overflow": overflow,
        "x": x,
        "b1": np.asarray(b1, dtype=np.float64),
        "has_b1": has_b1,
    }
    return in_maps, meta


def _finalize(results, meta, W1, W2, b2):
    """acc = sum_e u_e @ W2[e] (+ host ragged tail) + csum @ b2; /tw."""
    slot_exp = meta["slot_exp"]
    u = np.zeros((E, F), dtype=np.float64)
    for i in range(N_CORES):
        for s in range(EPC):
            u[slot_exp[s][i]] += results[i]["u"][s].astype(np.float64)
    for e, toks, w in meta["overflow"]:
        y = meta["x"][toks].astype(np.float64) @ W1[e].astype(np.float64)
        if meta["has_b1"]:
            y += meta["b1"][e]
        u[e] += w @ _gelu_exact(y)
    acc = np.einsum("ef,efh->h", u, W2.astype(np.float64))
    acc += meta["csum"] @ b2.astype(np.float64)
    return (acc / meta["total_weight"]).reshape(1, 1, H).astype(np.float32)


def kernel(input_tensor, Wg, bg, W1, b1, W2, b2):
    in_maps, meta = _prep_inputs(input_tensor, Wg, bg, W1, b1, W2, b2)
    nc = _build(meta["nts"], meta["has_b1"])
    res = run_bass_kernel_spmd(nc, in_maps, core_ids=list(range(N_CORES)))
    return _finalize(res.results, meta, W1, W2, b2)
